# revision 6
# baseline (speedup 1.0000x reference)
"""ContextAttention pooling kernel for 8 Trainium2 NeuronCores.

Reference computation (B=131072, L=1, Z=768, H=6 heads, hs=128):
    u_t    = tanh(x @ W.T + b)                       [B, 1, Z]
    scores = einsum('blhd,hd->hbl', u_t.reshape(B,L,H,hs), u_c)   [H, B]
    a      = softmax(scores, axis=-1)                [H, B]
    m[h]   = sum_b a[h, b] * x[b, 0, h*hs:(h+1)*hs]  -> [1, 1, Z]

Strategy:
  - Shard dim 0 (B) across 8 cores, 16384 rows each. Embarrassingly parallel.
  - tanh bounds |score| <= sum|u_c| ~= 64 < 88, so exp(score) never overflows
    fp32: use UNNORMALIZED softmax (no max pass, no cross-core reduce at all).
  - Each core computes partials m_unnorm[h, j] = sum_i exp(s[h,i]) x[i,j] and
    S[h] = sum_i exp(s[h,i]) over its rows.
  - Host sums the 8 partials, divides by S, extracts per-head diagonal blocks.

Per-core pipeline over 32 row-tiles of 512 rows:
  pass1: up[i,j] = sum_k x[i,k] W[j,k] + b[j]   (PE, f32r, x.T chunks stationary)
  tanh (ACT) -> multiply by u_c flat (DVE) -> segmented reduce_sum -> scores.T
  exp (ACT) -> w.T [128, 6] -> pass2 matmuls accumulate m_unnorm / S in PSUM.
"""

import numpy as np

B_FULL = 131072
Z = 768
H = 6
HS = 128
N_CORES = 8
BL = B_FULL // N_CORES        # rows per core = 16384
TILE = 512                    # rows per row-tile
NT = BL // TILE               # 32 row-tiles
NSUB = TILE // 128            # 4 sub-chunks of 128 rows
NK = Z // 128                 # 6 contraction chunks

_CACHE = {}


def _build():
    import concourse.bass as bass
    import concourse.tile as tile
    from concourse import bacc, mybir
    from contextlib import ExitStack

    F32 = mybir.dt.float32
    F32R = mybir.dt.float32r
    AF = mybir.ActivationFunctionType

    nc = bacc.Bacc("TRN2", target_bir_lowering=False, debug=False,
                   num_devices=N_CORES)

    x_d = nc.dram_tensor("x", [BL, Z], F32R, kind="ExternalInput").ap()
    wt_d = nc.dram_tensor("wt", [Z, Z], F32R, kind="ExternalInput").ap()  # W.T
    b_d = nc.dram_tensor("b", [1, Z], F32R, kind="ExternalInput").ap()
    uc_d = nc.dram_tensor("uc", [Z], F32, kind="ExternalInput").ap()      # u_c flat
    ones_d = nc.dram_tensor("ones", [1, 128], F32R, kind="ExternalInput").ap()
    out_d = nc.dram_tensor("out", [H, Z + 2], F32, kind="ExternalOutput").ap()

    with tile.TileContext(nc) as tc, ExitStack() as ctx:
        consts = ctx.enter_context(tc.tile_pool(name="consts", bufs=1))
        xtp = ctx.enter_context(tc.tile_pool(name="xt", bufs=3))
        xnp = ctx.enter_context(tc.tile_pool(name="xn", bufs=3))
        ttp = ctx.enter_context(tc.tile_pool(name="tt", bufs=3))
        tcp = ctx.enter_context(tc.tile_pool(name="tcu", bufs=3))
        scp = ctx.enter_context(tc.tile_pool(name="sc", bufs=6))
        oop = ctx.enter_context(tc.tile_pool(name="oo", bufs=1))
        psp = ctx.enter_context(tc.tile_pool(name="ps", bufs=2, space="PSUM"))
        accp = ctx.enter_context(tc.tile_pool(name="acc", bufs=1, space="PSUM"))

        # W.T resident: wt_sb[p, kk, j] = W[j, kk*128+p]
        wt_sb = consts.tile([128, NK, Z], F32R)
        nc.sync.dma_start(out=wt_sb,
                          in_=wt_d.rearrange("(kk p) j -> p kk j", p=128))
        # u_c flattened, broadcast to all 128 partitions
        ucb_sb = consts.tile([128, Z], F32)
        uc_bcast = bass.AP(tensor=uc_d.tensor, offset=uc_d.offset,
                           ap=[[0, 128]] + [list(d) for d in uc_d.ap])
        nc.sync.dma_start(out=ucb_sb, in_=uc_bcast)
        # bias row [1, Z] + ones row [1, 128] for the K=1 bias matmul;
        # ones col [128, 2] as moving operand for the S accumulation matmul
        b_sb = consts.tile([1, Z], F32R)
        nc.sync.dma_start(out=b_sb, in_=b_d)
        ones_sb = consts.tile([1, 128], F32R)
        nc.sync.dma_start(out=ones_sb, in_=ones_d)
        onescol_sb = consts.tile([128, 2], F32R)
        ones_bcast = bass.AP(tensor=ones_d.tensor, offset=ones_d.offset,
                             ap=[[0, 128], [1, 2]])
        nc.sync.dma_start(out=onescol_sb, in_=ones_bcast)

        # persistent PSUM accumulators: m_unnorm in m1|m2, S in ms
        m1 = accp.tile([H, 384], F32)
        m2 = accp.tile([H, 384], F32)
        ms = accp.tile([H, 2], F32)

        for r in range(NT):
            rows = x_d[r * TILE:(r + 1) * TILE, :]
            # transposed view for pass1 stationary: xt[p, kk, f] = x[i0+f, kk*128+p]
            xt = xtp.tile([128, NK, TILE], F32R)
            xt_src = rows.rearrange("f (kk p) -> p kk f", p=128)
            for kk in range(NK):
                nc.sync.dma_start(out=xt[:, kk, :], in_=xt_src[:, kk, :])
            # natural view for pass2 moving operand
            xn = xnp.tile([128, NSUB, Z], F32R)
            xn_src = rows.rearrange("(s p) j -> p s j", p=128)
            for s in range(NSUB):
                nc.sync.dma_start(out=xn[:, s, :], in_=xn_src[:, s, :])

            for s in range(NSUB):
                # pass1: up[i, j] = sum_k x[i,k] W[j,k] + b[j], [128, 768] in 2 banks
                up = psp.tile([128, 2, 384], F32, padded_shape=[128, 2, 512])
                for kk in range(NK):
                    lhsT = xt[:, kk, s * 128:(s + 1) * 128]
                    nc.tensor.matmul(up[:, 0, :], lhsT=lhsT,
                                     rhs=wt_sb[:, kk, 0:384],
                                     start=(kk == 0), stop=False)
                    nc.tensor.matmul(up[:, 1, :], lhsT=lhsT,
                                     rhs=wt_sb[:, kk, 384:768],
                                     start=(kk == 0), stop=False)
                nc.tensor.matmul(up[:, 0, :], lhsT=ones_sb,
                                 rhs=b_sb[:, 0:384], start=False, stop=True)
                nc.tensor.matmul(up[:, 1, :], lhsT=ones_sb,
                                 rhs=b_sb[:, 384:768], start=False, stop=True)

                # t = tanh(up)  [128, 768] (ACT, PSUM -> SBUF)
                tt = ttp.tile([128, 2, 384], F32)
                nc.scalar.activation(out=tt, in_=up, func=AF.Tanh)
                # tcu = t * u_c_flat ; scores.T[i, h] = sum_d tcu[i, h*128+d]
                tcu = tcp.tile([128, Z], F32)
                nc.vector.tensor_mul(tcu, tt.rearrange("p a b -> p (a b)"),
                                     ucb_sb)
                sraw = scp.tile([128, H], F32)
                nc.vector.reduce_sum(
                    out=sraw,
                    in_=tcu.rearrange("p (h d) -> p h d", h=H),
                    axis=mybir.AxisListType.X)
                # w.T = exp(scores.T)  [128, 6], produced as f32r for matmul
                wexp = scp.tile([128, H], F32R)
                nc.scalar.activation(out=wexp, in_=sraw, func=AF.Exp)

                # pass2: m_unnorm[h, j] += sum_i w[i,h] x[i,j];  S[h] += sum_i w[i,h]
                first = (r == 0 and s == 0)
                last = (r == NT - 1 and s == NSUB - 1)
                nc.tensor.matmul(m1, lhsT=wexp, rhs=xn[:, s, 0:384],
                                 start=first, stop=last)
                nc.tensor.matmul(m2, lhsT=wexp, rhs=xn[:, s, 384:768],
                                 start=first, stop=last)
                nc.tensor.matmul(ms, lhsT=wexp, rhs=onescol_sb,
                                 start=first, stop=last)

        ob = oop.tile([H, Z + 2], F32)
        nc.vector.tensor_copy(out=ob[:, 0:384], in_=m1)
        nc.vector.tensor_copy(out=ob[:, 384:768], in_=m2)
        nc.vector.tensor_copy(out=ob[:, 768:770], in_=ms)
        nc.sync.dma_start(out=out_d, in_=ob)

    nc.compile()
    return nc


def _get_nc():
    if "nc" not in _CACHE:
        _CACHE["nc"] = _build()
    return _CACHE["nc"]


def kernel(x, W, b, u_c):
    """x [131072, 1, 768] f32, W [768, 768] f32, b [768] f32, u_c [6, 128, 1] f32
    -> [1, 1, 768] f32"""
    from concourse.bass_utils import run_bass_kernel_spmd

    nc = _get_nc()

    x2 = np.ascontiguousarray(np.asarray(x).reshape(B_FULL, Z), dtype=np.float32)
    wt = np.ascontiguousarray(np.asarray(W).T, dtype=np.float32)
    bb = np.ascontiguousarray(np.asarray(b).reshape(1, Z), dtype=np.float32)
    uc = np.ascontiguousarray(np.asarray(u_c).reshape(Z), dtype=np.float32)
    ones = np.ones((1, 128), dtype=np.float32)

    in_maps = [
        {"x": x2[c * BL:(c + 1) * BL], "wt": wt, "b": bb, "uc": uc, "ones": ones}
        for c in range(N_CORES)
    ]
    res = run_bass_kernel_spmd(nc, in_maps, list(range(N_CORES)))

    tot = np.zeros((H, Z + 2), dtype=np.float64)
    for r in res.results:
        tot += r["out"].astype(np.float64)
    S = tot[:, Z]
    m = tot[:, :Z]
    out = np.empty((Z,), dtype=np.float64)
    for h in range(H):
        out[h * HS:(h + 1) * HS] = m[h, h * HS:(h + 1) * HS] / S[h]
    return out.reshape(1, 1, Z).astype(np.float32)


# revision 9
# speedup vs baseline: 104.5236x; 104.5236x over previous
"""ContextAttention pooling kernel for 8 Trainium2 NeuronCores.

Reference computation (B=131072, L=1, Z=768, H=6 heads, hs=128):
    u_t    = tanh(x @ W.T + b)                       [B, 1, Z]
    scores = einsum('blhd,hd->hbl', u_t.reshape(B,L,H,hs), u_c)   [H, B]
    a      = softmax(scores, axis=-1)                [H, B]
    m[h]   = sum_b a[h, b] * x[b, 0, h*hs:(h+1)*hs]  -> [1, 1, Z]

Strategy:
  - Shard dim 0 (B) across 8 cores, 16384 rows each. Embarrassingly parallel.
  - tanh bounds |score| <= sum|u_c| ~= 64 < 88, so exp(score) never overflows
    fp32: use UNNORMALIZED softmax (no max pass, no cross-core reduce at all).
  - Each core computes partials m_unnorm[h, j] = sum_i exp(s[h,i]) x[i,j] and
    S[h] = sum_i exp(s[h,i]) over its rows.
  - Host sums the 8 partials, divides by S, extracts per-head diagonal blocks.

Per-core pipeline over 32 row-tiles of 512 rows:
  pass1: up[i,j] = sum_k x[i,k] W[j,k] + b[j]   (PE, f32r, x.T chunks stationary)
  tanh (ACT) -> multiply by u_c flat (DVE) -> segmented reduce_sum -> scores.T
  exp (ACT) -> w.T [128, 6] -> pass2 matmuls accumulate m_unnorm / S in PSUM.
"""

import numpy as np

B_FULL = 131072
Z = 768
H = 6
HS = 128
N_CORES = 8
BL = B_FULL // N_CORES        # rows per core = 16384
TILE = 512                    # rows per row-tile
NT = BL // TILE               # 32 row-tiles
NSUB = TILE // 128            # 4 sub-chunks of 128 rows
NK = Z // 128                 # 6 contraction chunks

_CACHE = {}


def _build():
    import concourse.bass as bass
    import concourse.tile as tile
    from concourse import bacc, mybir
    from contextlib import ExitStack

    F32 = mybir.dt.float32
    F32R = mybir.dt.float32r
    AF = mybir.ActivationFunctionType

    nc = bacc.Bacc("TRN2", target_bir_lowering=False, debug=False,
                   num_devices=N_CORES)

    x_d = nc.dram_tensor("x", [BL, Z], F32R, kind="ExternalInput").ap()
    xt_d = nc.dram_tensor("xT", [Z, BL], F32R, kind="ExternalInput").ap()  # x.T
    wt_d = nc.dram_tensor("wt", [Z, Z], F32R, kind="ExternalInput").ap()  # W.T
    b_d = nc.dram_tensor("b", [1, Z], F32R, kind="ExternalInput").ap()
    uc_d = nc.dram_tensor("uc", [Z], F32, kind="ExternalInput").ap()      # u_c flat
    ones_d = nc.dram_tensor("ones", [1, 128], F32R, kind="ExternalInput").ap()
    out_d = nc.dram_tensor("out", [H, Z + 2], F32, kind="ExternalOutput").ap()

    with tile.TileContext(nc) as tc, ExitStack() as ctx:
        consts = ctx.enter_context(tc.tile_pool(name="consts", bufs=1))
        xtp = ctx.enter_context(tc.tile_pool(name="xt", bufs=3))
        xnp = ctx.enter_context(tc.tile_pool(name="xn", bufs=3))
        ttp = ctx.enter_context(tc.tile_pool(name="tt", bufs=3))
        tcp = ctx.enter_context(tc.tile_pool(name="tcu", bufs=3))
        scp = ctx.enter_context(tc.tile_pool(name="sc", bufs=6))
        oop = ctx.enter_context(tc.tile_pool(name="oo", bufs=1))
        psp = ctx.enter_context(tc.tile_pool(name="ps", bufs=2, space="PSUM"))
        accp = ctx.enter_context(tc.tile_pool(name="acc", bufs=1, space="PSUM"))

        # W.T resident: wt_sb[p, kk, j] = W[j, kk*128+p]
        wt_sb = consts.tile([128, NK, Z], F32R)
        nc.sync.dma_start(out=wt_sb,
                          in_=wt_d.rearrange("(kk p) j -> p kk j", p=128))
        # u_c flattened, broadcast to all 128 partitions
        ucb_sb = consts.tile([128, Z], F32)
        uc_bcast = bass.AP(tensor=uc_d.tensor, offset=uc_d.offset,
                           ap=[[0, 128]] + [list(d) for d in uc_d.ap])
        nc.sync.dma_start(out=ucb_sb, in_=uc_bcast)
        # bias row [1, Z] + ones row [1, 128] for the K=1 bias matmul;
        # ones col [128, 2] as moving operand for the S accumulation matmul
        b_sb = consts.tile([1, Z], F32R)
        nc.sync.dma_start(out=b_sb, in_=b_d)
        ones_sb = consts.tile([1, 128], F32R)
        nc.sync.dma_start(out=ones_sb, in_=ones_d)
        onescol_sb = consts.tile([128, 2], F32R)
        ones_bcast = bass.AP(tensor=ones_d.tensor, offset=ones_d.offset,
                             ap=[[0, 128], [1, 2]])
        nc.sync.dma_start(out=onescol_sb, in_=ones_bcast)

        # persistent PSUM accumulators: m_unnorm in m1|m2, S in ms
        m1 = accp.tile([H, 384], F32)
        m2 = accp.tile([H, 384], F32)
        ms = accp.tile([H, 2], F32)

        for r in range(NT):
            # transposed view for pass1 stationary: xt[p, kk, f] = x[i0+f, kk*128+p]
            # (from the host-pretransposed copy -> contiguous per-partition reads)
            xt = xtp.tile([128, NK, TILE], F32R)
            xt_src = xt_d[:, r * TILE:(r + 1) * TILE].rearrange(
                "(kk p) f -> p kk f", p=128)
            nc.sync.dma_start(out=xt, in_=xt_src)
            # natural view for pass2 moving operand
            xn = xnp.tile([128, NSUB, Z], F32R)
            xn_src = x_d[r * TILE:(r + 1) * TILE, :].rearrange(
                "(s p) j -> p s j", p=128)
            nc.sync.dma_start(out=xn, in_=xn_src)

            for s in range(NSUB):
                # pass1: up[i, j] = sum_k x[i,k] W[j,k] + b[j], [128, 768] in 2 banks
                up = psp.tile([128, 2, 384], F32, padded_shape=[128, 2, 512])
                for kk in range(NK):
                    lhsT = xt[:, kk, s * 128:(s + 1) * 128]
                    nc.tensor.matmul(up[:, 0, :], lhsT=lhsT,
                                     rhs=wt_sb[:, kk, 0:384],
                                     start=(kk == 0), stop=False)
                    nc.tensor.matmul(up[:, 1, :], lhsT=lhsT,
                                     rhs=wt_sb[:, kk, 384:768],
                                     start=(kk == 0), stop=False)
                nc.tensor.matmul(up[:, 0, :], lhsT=ones_sb,
                                 rhs=b_sb[:, 0:384], start=False, stop=True)
                nc.tensor.matmul(up[:, 1, :], lhsT=ones_sb,
                                 rhs=b_sb[:, 384:768], start=False, stop=True)

                # t = tanh(up)  [128, 768] (ACT, PSUM -> SBUF)
                tt = ttp.tile([128, 2, 384], F32)
                nc.scalar.activation(out=tt, in_=up, func=AF.Tanh)
                # tcu = t * u_c_flat ; scores.T[i, h] = sum_d tcu[i, h*128+d]
                tcu = tcp.tile([128, Z], F32)
                nc.vector.tensor_mul(tcu, tt.rearrange("p a b -> p (a b)"),
                                     ucb_sb)
                sraw = scp.tile([128, H], F32)
                nc.vector.reduce_sum(
                    out=sraw,
                    in_=tcu.rearrange("p (h d) -> p h d", h=H),
                    axis=mybir.AxisListType.X)
                # w.T = exp(scores.T)  [128, 6], produced as f32r for matmul
                wexp = scp.tile([128, H], F32R)
                nc.scalar.activation(out=wexp, in_=sraw, func=AF.Exp)

                # pass2: m_unnorm[h, j] += sum_i w[i,h] x[i,j];  S[h] += sum_i w[i,h]
                first = (r == 0 and s == 0)
                last = (r == NT - 1 and s == NSUB - 1)
                nc.tensor.matmul(m1, lhsT=wexp, rhs=xn[:, s, 0:384],
                                 start=first, stop=last)
                nc.tensor.matmul(m2, lhsT=wexp, rhs=xn[:, s, 384:768],
                                 start=first, stop=last)
                nc.tensor.matmul(ms, lhsT=wexp, rhs=onescol_sb,
                                 start=first, stop=last)

        ob = oop.tile([H, Z + 2], F32)
        nc.vector.tensor_copy(out=ob[:, 0:384], in_=m1)
        nc.vector.tensor_copy(out=ob[:, 384:768], in_=m2)
        nc.vector.tensor_copy(out=ob[:, 768:770], in_=ms)
        nc.sync.dma_start(out=out_d, in_=ob)

    nc.compile()
    return nc


def _get_nc():
    if "nc" not in _CACHE:
        _CACHE["nc"] = _build()
    return _CACHE["nc"]


def kernel(x, W, b, u_c):
    """x [131072, 1, 768] f32, W [768, 768] f32, b [768] f32, u_c [6, 128, 1] f32
    -> [1, 1, 768] f32"""
    from concourse.bass_utils import run_bass_kernel_spmd

    nc = _get_nc()

    x2 = np.ascontiguousarray(np.asarray(x).reshape(B_FULL, Z), dtype=np.float32)
    wt = np.ascontiguousarray(np.asarray(W).T, dtype=np.float32)
    bb = np.ascontiguousarray(np.asarray(b).reshape(1, Z), dtype=np.float32)
    uc = np.ascontiguousarray(np.asarray(u_c).reshape(Z), dtype=np.float32)
    ones = np.ones((1, 128), dtype=np.float32)

    in_maps = [
        {"x": x2[c * BL:(c + 1) * BL],
         "xT": np.ascontiguousarray(x2[c * BL:(c + 1) * BL].T),
         "wt": wt, "b": bb, "uc": uc, "ones": ones}
        for c in range(N_CORES)
    ]
    res = run_bass_kernel_spmd(nc, in_maps, list(range(N_CORES)))

    tot = np.zeros((H, Z + 2), dtype=np.float64)
    for r in res.results:
        tot += r["out"].astype(np.float64)
    S = tot[:, Z]
    m = tot[:, :Z]
    out = np.empty((Z,), dtype=np.float64)
    for h in range(H):
        out[h * HS:(h + 1) * HS] = m[h, h * HS:(h + 1) * HS] / S[h]
    return out.reshape(1, 1, Z).astype(np.float32)
